# revision 45
# baseline (speedup 1.0000x reference)
"""Trainium2 Bass kernel for nn_CodecAttention (sliding-window ALiBi attention).

Reference computation (B=4, T=2048, DIM=1024, H=8, HD=128, WINDOW=16):
    xq = rms_norm(x @ wq) ; xk = rms_norm(x @ wk) ; xv = x @ wv
    scores = q k^T / sqrt(HD) + alibi_bias  (causal + 16-token sliding window)
    out = softmax(scores) @ v  -> reshape -> @ wo

Sharding: 8 cores = (batch b, sequence half). Each core processes 1024 query
tokens plus a 128-token key/value halo (zeros for the first half), fully
locally -- the attention window (16) never crosses the halo, so no
collectives are needed.

Layout strategy (per core): everything transposed. Host passes xT [DIM, 1152].
Projections produce qT/kT in [dim, tok] layout and v in natural [tok, dim]
layout. Scores are computed transposed (sT[k, q] = kT.T @ qT per head), the
softmax denominator comes from a ones-column matmul (reduction over the
partition axis), and PV produces attn_outT[d, q] = v.T-free matmul with
exp(sT) as the moving operand. attn_outT is exactly the stationary operand the
wo matmul wants, so the final output lands in natural [tok, dim] layout with
zero transposes anywhere.

All matmuls run in float32r (full PE rate at N>=256, ~1.6e-4 rel err/K=128).
RMS norm: sum-of-squares via ones-matmul, rsqrt via ACT Sqrt + DVE reciprocal,
applied through a K=1 broadcast matmul (rstd per token broadcast across
partitions; the k-side broadcast uses q_norm_w*k_norm_w/sqrt(HD) as the
stationary operand, folding the norm weights and score scale in for free).

ALiBi + causal + window mask: tiny per-(j) rel tiles with -1e9 at invalid
positions; scores += slope_h * rel via one fused scalar_tensor_tensor. The
first key tile of the first q-chunk additionally subtracts a per-core
"negcol" column that kills out-of-range (global position < 0) halo keys.
"""

import math
import os

import numpy as np

os.environ.setdefault("MYCRO_LOCAL_CACHE", "1")

import concourse.mybir as mybir
import concourse.tile as tile
from concourse import bacc
from concourse.bass_utils import run_bass_kernel_spmd

F32 = mybir.dt.float32
F32R = mybir.dt.float32r
AF = mybir.ActivationFunctionType
ALU = mybir.AluOpType

B, T, DIM = 4, 2048, 1024
H, HD = 8, 128
WINDOW = 16
EPS = 1e-6
NEG = -1.0e9
BIGMASK = 30000.0

HALO = 128                 # key/value halo tokens per shard
TSH = HALO + T // 2        # 1152 tokens per shard
QTOK = T // 2              # 1024 query tokens per shard
ND = DIM // 128            # 8 dim tiles
NT = TSH // 128            # 9 token tiles
QC = 256                   # attention query-chunk width
NQC = QTOK // QC           # 4 query chunks
K_CHUNKS = [(0, 384), (384, 384), (768, 384)]        # kT projection chunks
Q_CHUNKS = [(0, 512), (512, 512)]                    # qT projection chunks

_SLOPES = [2.0 ** (-i) for i in range(H)]

_CACHE = {}


def _build_program():
    nc = bacc.Bacc("TRN2", debug=False, target_bir_lowering=False, num_devices=8)

    xt = nc.declare_dram_parameter("xt", [128, ND, TSH], F32R, isOutput=False)
    wq = nc.declare_dram_parameter("wq", [DIM, DIM], F32R, isOutput=False)
    wk = nc.declare_dram_parameter("wk", [DIM, DIM], F32R, isOutput=False)
    wv = nc.declare_dram_parameter("wv", [DIM, DIM], F32R, isOutput=False)
    wo = nc.declare_dram_parameter("wo", [DIM, DIM], F32R, isOutput=False)
    qkw_row = nc.declare_dram_parameter("qkw_row", [1, ND, 128], F32R, isOutput=False)
    ones_row = nc.declare_dram_parameter("ones_row", [1, 128], F32R, isOutput=False)
    ones_col = nc.declare_dram_parameter("ones_col", [128, 1], F32R, isOutput=False)
    rel3 = nc.declare_dram_parameter("rel3", [128, 3, QC], F32, isOutput=False)
    negcol = nc.declare_dram_parameter("negcol", [128, 1], F32, isOutput=False)
    out = nc.declare_dram_parameter("out", [QTOK, DIM], F32, isOutput=True)

    with tile.TileContext(nc) as tc:
        with tc.tile_pool(name="big", bufs=1) as big:
            # ---- constants + persistent tensors (live for the whole kernel) ----
            kt_sb = big.tile([128, ND, TSH], F32R)
            qt_sb = big.tile([128, ND, QTOK], F32R)
            v_sb = big.tile([128, NT, DIM], F32R)
            qkw_sb = big.tile([1, ND, 128], F32R)
            onesr_sb = big.tile([1, 128], F32R)
            onesc_sb = big.tile([128, 1], F32R)
            rel3_sb = big.tile([128, 3, QC], F32)
            negcol_sb = big.tile([128, 1], F32)
            eps_sb = big.tile([1, 1], F32)
            nc.vector.memset(eps_sb[:], EPS)
            nc.sync.dma_start(qkw_sb[:], qkw_row[:])
            nc.sync.dma_start(onesr_sb[:], ones_row[:])
            nc.sync.dma_start(onesc_sb[:], ones_col[:])
            nc.sync.dma_start(rel3_sb[:], rel3[:])
            nc.sync.dma_start(negcol_sb[:], negcol[:])

            self_phase1(tc, nc, kt_sb, qt_sb, v_sb, qkw_sb, onesr_sb, onesc_sb,
                        eps_sb, xt, wq, wk, wv)
            self_phase2(tc, nc, kt_sb, qt_sb, v_sb, onesr_sb, onesc_sb,
                        rel3_sb, negcol_sb, wo, out)
    nc.compile()
    return nc


def self_phase1(tc, nc, kt_sb, qt_sb, v_sb, qkw_sb, onesr_sb, onesc_sb,
                eps_sb, xt, wq, wk, wv):
    with (
        tc.tile_pool(name="xtp", bufs=1) as xtp,
        tc.tile_pool(name="wp", bufs=int(os.environ.get("KP_WP", 9))) as wp,
        tc.tile_pool(name="scr", bufs=2) as scrp,
        tc.tile_pool(name="sqt", bufs=1) as sqtp,
        tc.tile_pool(name="rcp", bufs=2) as rcpp,
        tc.tile_pool(name="pp", bufs=int(os.environ.get("KP_PP", 6)),
                     space="PSUM") as pp,
        tc.tile_pool(name="sqp", bufs=int(os.environ.get("KP_SQP", 1)),
                     space="PSUM") as sqp,
        tc.tile_pool(name="bcp", bufs=int(os.environ.get("KP_BCP", 1)),
                     space="PSUM") as bcp,
    ):
            xt_sb = xtp.tile([128, ND, TSH], F32R)

            # ---- projections: kT and qT (with RMS-norm), v (plain) ----
            def drain_ps(dst, ps, m, c0, cw, ssq):
                # raw copy (rounded to f32r) + square + ssq accumulation;
                # alternate engines to balance ACT vs DVE load
                if m % 2 == 0:
                    nc.scalar.copy(dst[:, m, c0:c0 + cw], ps[:, :cw])
                else:
                    nc.vector.tensor_copy(dst[:, m, c0:c0 + cw], ps[:, :cw])
                sq = scrp.tile([128, 512], F32R, tag="sq")
                if m % 2 == 0:
                    # DVE square must read the SBUF copy (one-PSUM-input rule)
                    nc.vector.tensor_mul(sq[:, :cw], dst[:, m, c0:c0 + cw],
                                         dst[:, m, c0:c0 + cw])
                else:
                    nc.scalar.square(sq[:, :cw], ps[:, :cw])
                nc.tensor.matmul(
                    ssq[:, :cw], onesc_sb[:], sq[:, :cw],
                    start=(m == 0), stop=(m == ND - 1),
                )

            def proj_normed(w_dram, dst, chunks, tok0, fold_qkw, first=False,
                            pool=None):
                pool = pool or pp
                """dst[:, m, c] = rstd * (x @ w)^T, rstd from raw sum-of-squares."""
                w_slices = []
                for kk in range(ND):
                    w_sl = wp.tile([128, DIM], F32R, tag="wslice")
                    nc.sync.dma_start(w_sl[:], w_dram[kk * 128:(kk + 1) * 128, :])
                    if first:
                        # interleave xt loads so the kk-outer first chunk can
                        # start as soon as the first (w, xt) slice pair lands
                        nc.sync.dma_start(xt_sb[:, kk, :], xt[:, kk, :])
                    w_slices.append(w_sl)
                for ci, (c0, cw) in enumerate(chunks):
                    ssq = sqp.tile([1, 512], F32)
                    if first and ci == 0:
                        # kk-outer in m-blocks of 4: PE consumes DMA'd slices
                        # incrementally instead of waiting for all 16
                        for mb in range(0, ND, 4):
                            blk = []
                            for m in range(mb, mb + 4):
                                ps = pool.tile([128, 512], F32, tag="ps")
                                blk.append(ps)
                            for kk in range(ND):
                                for mi, m in enumerate(range(mb, mb + 4)):
                                    nc.tensor.matmul(
                                        blk[mi][:, :cw],
                                        w_slices[kk][:, m * 128:(m + 1) * 128],
                                        xt_sb[:, kk, tok0 + c0: tok0 + c0 + cw],
                                        start=(kk == 0), stop=(kk == ND - 1),
                                    )
                            for mi, m in enumerate(range(mb, mb + 4)):
                                drain_ps(dst, blk[mi], m, c0, cw, ssq)
                    else:
                        for m in range(ND):
                            ps = pool.tile([128, 512], F32, tag="ps")
                            for kk in range(ND):
                                nc.tensor.matmul(
                                    ps[:, :cw],
                                    w_slices[kk][:, m * 128:(m + 1) * 128],
                                    xt_sb[:, kk, tok0 + c0: tok0 + c0 + cw],
                                    start=(kk == 0), stop=(kk == ND - 1),
                                )
                            drain_ps(dst, ps, m, c0, cw, ssq)
                    sqt = sqtp.tile([1, 512], F32, tag="sqt")
                    nc.scalar.activation(sqt[:, :cw], ssq[:, :cw], AF.Sqrt,
                                         bias=eps_sb[:], scale=1.0 / DIM)
                    rstd = rcpp.tile([1, 512], F32R, tag="rstd")
                    with nc.allow_low_precision(reason="f32r rstd for matmul"):
                        nc.vector.reciprocal(rstd[:, :cw], sqt[:, :cw])
                    if fold_qkw:
                        for m in range(ND):
                            rsb = bcp.tile([128, 512], F32)
                            nc.tensor.matmul(rsb[:, :cw], qkw_sb[:, m, :],
                                             rstd[:, :cw], start=True, stop=True)
                            nc.vector.tensor_mul(dst[:, m, c0:c0 + cw],
                                                 dst[:, m, c0:c0 + cw], rsb[:, :cw])
                    else:
                        rsb = bcp.tile([128, 512], F32)
                        nc.tensor.matmul(rsb[:, :cw], onesr_sb[:],
                                         rstd[:, :cw], start=True, stop=True)
                        for m in range(ND):
                            nc.vector.tensor_mul(dst[:, m, c0:c0 + cw],
                                                 dst[:, m, c0:c0 + cw], rsb[:, :cw])

            proj_normed(wk, kt_sb, K_CHUNKS, 0, fold_qkw=True, first=True)
            proj_normed(wq, qt_sb, Q_CHUNKS, HALO, fold_qkw=False)

            # v: natural layout [tok, dim]
            wv_slices = []
            for kk in range(ND):
                w_sl = wp.tile([128, DIM], F32R, tag="wslice")
                nc.sync.dma_start(w_sl[:], wv[kk * 128:(kk + 1) * 128, :])
                wv_slices.append(w_sl)
            for tt in range(NT):
                for nn in range(2):
                    ps = pp.tile([128, 512], F32)
                    for kk in range(ND):
                        nc.tensor.matmul(
                            ps[:],
                            xt_sb[:, kk, tt * 128:(tt + 1) * 128],
                            wv_slices[kk][:, nn * 512:(nn + 1) * 512],
                            start=(kk == 0), stop=(kk == ND - 1),
                        )
                    if tt % 2 == 0:
                        nc.scalar.copy(v_sb[:, tt, nn * 512:(nn + 1) * 512], ps[:])
                    else:
                        nc.vector.tensor_copy(v_sb[:, tt, nn * 512:(nn + 1) * 512],
                                              ps[:])


def self_phase2(tc, nc, kt_sb, qt_sb, v_sb, onesr_sb, onesc_sb,
                rel3_sb, negcol_sb, wo, out):
        # xt freed; load wo and run attention + output projection
        with (
            tc.tile_pool(name="wo", bufs=1) as wop,
            tc.tile_pool(name="exp", bufs=int(os.environ.get("KP_EXP", 3))) as expp,
            tc.tile_pool(name="atc", bufs=int(os.environ.get("KP_ATC", 2))) as atcp,
            tc.tile_pool(name="outp", bufs=3) as outp,
            tc.tile_pool(name="rcp2", bufs=2) as rcp2p,
            tc.tile_pool(name="sps", bufs=int(os.environ.get("KP_SPS", 2)),
                         space="PSUM") as sps,
            tc.tile_pool(name="ytp", bufs=int(os.environ.get("KP_YTP", 1)),
                         space="PSUM") as ytp,
            tc.tile_pool(name="rsp", bufs=int(os.environ.get("KP_RSP", 1)),
                         space="PSUM") as rsp,
            tc.tile_pool(name="bc2", bufs=int(os.environ.get("KP_BC2", 1)),
                         space="PSUM") as bc2p,
            tc.tile_pool(name="pso", bufs=int(os.environ.get("KP_PSO", 1)),
                         space="PSUM") as psop,
        ):
            wo_sb = wop.tile([128, ND, DIM], F32R)
            for hd in range(ND):
                nc.sync.dma_start(wo_sb[:, hd, :], wo[hd * 128:(hd + 1) * 128, :])

            for qc in range(NQC):
                aT = atcp.tile([128, ND, QC], F32R)
                for h in range(H):
                    yT_t = ytp.tile([128, QC], F32, tag="yT")
                    rs_t = rsp.tile([1, QC], F32, tag="rs")
                    yT = yT_t[:, :]
                    rs = rs_t[:, :]
                    # joint [128, 3, QC] score tile: three QK matmuls, then ONE
                    # fused bias-add and ONE exp over all 768 columns
                    stj = sps.tile([128, 3, QC], F32)
                    st = stj[:, 0:3, :]
                    for j in range(3):
                        nc.tensor.matmul(
                            stj[:, j, :],
                            kt_sb[:, h, qc * QC + j * 128: qc * QC + (j + 1) * 128],
                            qt_sb[:, h, qc * QC: (qc + 1) * QC],
                            start=True, stop=True,
                        )
                    if qc == 0:
                        nc.vector.tensor_scalar(
                            out=stj[:, 0, :], in0=stj[:, 0, :], scalar1=negcol_sb[:],
                            scalar2=None, op0=ALU.subtract)
                    # scores += slope_h * rel (rel = -1e9 at masked positions)
                    nc.vector.scalar_tensor_tensor(
                        out=st[:], in0=rel3_sb[:], scalar=_SLOPES[h],
                        in1=st[:], op0=ALU.mult, op1=ALU.add)
                    ex = expp.tile([128, 3, QC], F32R, tag="exp")
                    nc.scalar.activation(ex[:], st[:], AF.Exp)
                    for j in range(3):
                        nc.tensor.matmul(
                            yT,
                            v_sb[:, 2 * qc + j, h * 128:(h + 1) * 128],
                            ex[:, j, :], start=(j == 0), stop=(j == 2),
                        )
                        nc.tensor.matmul(
                            rs, onesc_sb[:], ex[:, j, :],
                            start=(j == 0), stop=(j == 2),
                        )
                    rcp = rcp2p.tile([1, QC], F32R, tag="rcp")
                    with nc.allow_low_precision(reason="f32r prob scale"):
                        nc.vector.reciprocal(rcp[:], rs)
                    rsb2_t = bc2p.tile([128, QC], F32, tag="rsb2")
                    rsb2 = rsb2_t[:, :]
                    nc.tensor.matmul(rsb2, onesr_sb[:], rcp[:],
                                     start=True, stop=True)
                    nc.scalar.copy(aT[:, h, :], yT)
                    nc.vector.tensor_mul(aT[:, h, :], aT[:, h, :], rsb2)

                # output projection for this q-chunk
                for t2 in range(QC // 128):
                    for nn in range(2):
                        ps_o = psop.tile([128, 512], F32)
                        for hd in range(ND):
                            nc.tensor.matmul(
                                ps_o[:],
                                aT[:, hd, t2 * 128:(t2 + 1) * 128],
                                wo_sb[:, hd, nn * 512:(nn + 1) * 512],
                                start=(hd == 0), stop=(hd == ND - 1),
                            )
                        o_sb = outp.tile([128, 512], F32, tag="osb")
                        nc.vector.tensor_copy(o_sb[:], ps_o[:])
                        nc.sync.dma_start(
                            out[qc * QC + t2 * 128: qc * QC + (t2 + 1) * 128,
                                nn * 512:(nn + 1) * 512],
                            o_sb[:],
                        )


def _host_constants():
    # rel3[kj, j, qi] = 128*(j-1) + kj - qi if in window else NEG
    kj = np.arange(128)[:, None, None]
    jj = np.arange(3)[None, :, None]
    qi = np.arange(QC)[None, None, :]
    rel = 128 * (jj - 1) + kj - qi
    valid = (rel <= 0) & (rel >= -WINDOW)
    rel3 = np.where(valid, rel, NEG).astype(np.float32)
    ones_row = np.ones((1, 128), dtype=np.float32)
    ones_col = np.ones((128, 1), dtype=np.float32)
    return rel3, ones_row, ones_col


def kernel(x, wq, wk, wv, wo, q_norm_w, k_norm_w):
    x = np.ascontiguousarray(np.asarray(x, dtype=np.float32))
    wq = np.ascontiguousarray(np.asarray(wq, dtype=np.float32))
    wk = np.ascontiguousarray(np.asarray(wk, dtype=np.float32))
    wv = np.ascontiguousarray(np.asarray(wv, dtype=np.float32))
    wo = np.ascontiguousarray(np.asarray(wo, dtype=np.float32))
    q_norm_w = np.asarray(q_norm_w, dtype=np.float32)
    k_norm_w = np.asarray(k_norm_w, dtype=np.float32)

    if "nc" not in _CACHE:
        _CACHE["nc"] = _build_program()
    nc = _CACHE["nc"]

    rel3, ones_row, ones_col = _host_constants()
    qkw = (q_norm_w * k_norm_w / math.sqrt(HD)).astype(np.float32)
    qkw_row = qkw.reshape(1, ND, 128)

    in_maps = []
    for c in range(8):
        b, hf = c // 2, c % 2
        base = hf * (T // 2)
        xsh = np.zeros((TSH, DIM), dtype=np.float32)
        lo = base - HALO
        if lo < 0:
            xsh[HALO:] = x[b, base: base + QTOK]
        else:
            xsh[:] = x[b, lo: base + QTOK]
        xt_c = np.ascontiguousarray(
            xsh.T.reshape(ND, 128, TSH).transpose(1, 0, 2))
        negcol = np.full((128, 1), BIGMASK if hf == 0 else 0.0, dtype=np.float32)
        in_maps.append({
            "xt": xt_c, "wq": wq, "wk": wk, "wv": wv, "wo": wo,
            "qkw_row": qkw_row, "ones_row": ones_row, "ones_col": ones_col,
            "rel3": rel3, "negcol": negcol,
        })

    _CACHE["in_maps"] = in_maps
    import time as _time
    last_err = None
    for attempt in range(3):
        try:
            res = run_bass_kernel_spmd(nc, in_maps, core_ids=list(range(8)))
            break
        except Exception as e:  # transient NRT/device wedges recover on retry
            last_err = e
            _time.sleep(10 * (attempt + 1))
    else:
        raise last_err

    out = np.empty((B, T, DIM), dtype=np.float32)
    for c in range(8):
        b, hf = c // 2, c % 2
        out[b, hf * QTOK:(hf + 1) * QTOK, :] = res.results[c]["out"]
    return out


# revision 47
# speedup vs baseline: 1.0233x; 1.0233x over previous
"""Trainium2 Bass kernel for nn_CodecAttention (sliding-window ALiBi attention).

Reference computation (B=4, T=2048, DIM=1024, H=8, HD=128, WINDOW=16):
    xq = rms_norm(x @ wq) ; xk = rms_norm(x @ wk) ; xv = x @ wv
    scores = q k^T / sqrt(HD) + alibi_bias  (causal + 16-token sliding window)
    out = softmax(scores) @ v  -> reshape -> @ wo

Sharding: 8 cores = (batch b, sequence half). Each core processes 1024 query
tokens plus a 128-token key/value halo (zeros for the first half), fully
locally -- the attention window (16) never crosses the halo, so no
collectives are needed.

Layout strategy (per core): everything transposed. Host passes xT [DIM, 1152].
Projections produce qT/kT in [dim, tok] layout and v in natural [tok, dim]
layout. Scores are computed transposed (sT[k, q] = kT.T @ qT per head), the
softmax denominator comes from a ones-column matmul (reduction over the
partition axis), and PV produces attn_outT[d, q] = v.T-free matmul with
exp(sT) as the moving operand. attn_outT is exactly the stationary operand the
wo matmul wants, so the final output lands in natural [tok, dim] layout with
zero transposes anywhere.

All matmuls run in float32r (full PE rate at N>=256, ~1.6e-4 rel err/K=128).
RMS norm: sum-of-squares via ones-matmul, rsqrt via ACT Sqrt + DVE reciprocal,
applied through a K=1 broadcast matmul (rstd per token broadcast across
partitions; the k-side broadcast uses q_norm_w*k_norm_w/sqrt(HD) as the
stationary operand, folding the norm weights and score scale in for free).

ALiBi + causal + window mask: tiny per-(j) rel tiles with -1e9 at invalid
positions; scores += slope_h * rel via one fused scalar_tensor_tensor. The
first key tile of the first q-chunk additionally subtracts a per-core
"negcol" column that kills out-of-range (global position < 0) halo keys.
"""

import math
import os

import numpy as np

os.environ.setdefault("MYCRO_LOCAL_CACHE", "1")

import concourse.mybir as mybir
import concourse.tile as tile
from concourse import bacc
from concourse.bass_utils import run_bass_kernel_spmd

F32 = mybir.dt.float32
F32R = mybir.dt.float32r
AF = mybir.ActivationFunctionType
ALU = mybir.AluOpType

B, T, DIM = 4, 2048, 1024
H, HD = 8, 128
WINDOW = 16
EPS = 1e-6
NEG = -1.0e9
BIGMASK = 30000.0

HALO = 128                 # key/value halo tokens per shard
TSH = HALO + T // 2        # 1152 tokens per shard
QTOK = T // 2              # 1024 query tokens per shard
ND = DIM // 128            # 8 dim tiles
NT = TSH // 128            # 9 token tiles
QC = 256                   # attention query-chunk width
NQC = QTOK // QC           # 4 query chunks
K_CHUNKS = [(0, 384), (384, 384), (768, 384)]        # kT projection chunks
Q_CHUNKS = [(0, 512), (512, 512)]                    # qT projection chunks

_SLOPES = [2.0 ** (-i) for i in range(H)]

_CACHE = {}


def _build_program():
    nc = bacc.Bacc("TRN2", debug=False, target_bir_lowering=False, num_devices=8)

    xt = nc.declare_dram_parameter("xt", [128, ND, TSH], F32R, isOutput=False)
    wq = nc.declare_dram_parameter("wq", [DIM, DIM], F32R, isOutput=False)
    wk = nc.declare_dram_parameter("wk", [DIM, DIM], F32R, isOutput=False)
    wv = nc.declare_dram_parameter("wv", [DIM, DIM], F32R, isOutput=False)
    wo = nc.declare_dram_parameter("wo", [DIM, DIM], F32R, isOutput=False)
    qkw_row = nc.declare_dram_parameter("qkw_row", [1, ND, 128], F32R, isOutput=False)
    ones_row = nc.declare_dram_parameter("ones_row", [1, 128], F32R, isOutput=False)
    ones_col = nc.declare_dram_parameter("ones_col", [128, 1], F32R, isOutput=False)
    rel4 = nc.declare_dram_parameter("rel4", [128, 4, QC], F32, isOutput=False)
    out = nc.declare_dram_parameter("out", [QTOK, DIM], F32, isOutput=True)

    with tile.TileContext(nc) as tc:
        with tc.tile_pool(name="big", bufs=1) as big:
            # ---- constants + persistent tensors (live for the whole kernel) ----
            kt_sb = big.tile([128, ND, TSH], F32R)
            qt_sb = big.tile([128, ND, QTOK], F32R)
            v_sb = big.tile([128, NT, DIM], F32R)
            qkw_sb = big.tile([1, ND, 128], F32R)
            onesr_sb = big.tile([1, 128], F32R)
            onesc_sb = big.tile([128, 1], F32R)
            rel4_sb = big.tile([128, 4, QC], F32)
            eps_sb = big.tile([1, 1], F32)
            nc.vector.memset(eps_sb[:], EPS)
            nc.sync.dma_start(qkw_sb[:], qkw_row[:])
            nc.sync.dma_start(onesr_sb[:], ones_row[:])
            nc.sync.dma_start(onesc_sb[:], ones_col[:])
            nc.sync.dma_start(rel4_sb[:], rel4[:])

            self_phase1(tc, nc, kt_sb, qt_sb, v_sb, qkw_sb, onesr_sb, onesc_sb,
                        eps_sb, xt, wq, wk, wv)
            self_phase2(tc, nc, kt_sb, qt_sb, v_sb, onesr_sb, onesc_sb,
                        rel4_sb, wo, out)
    nc.compile()
    return nc


def self_phase1(tc, nc, kt_sb, qt_sb, v_sb, qkw_sb, onesr_sb, onesc_sb,
                eps_sb, xt, wq, wk, wv):
    with (
        tc.tile_pool(name="xtp", bufs=1) as xtp,
        tc.tile_pool(name="wp", bufs=int(os.environ.get("KP_WP", 9))) as wp,
        tc.tile_pool(name="scr", bufs=2) as scrp,
        tc.tile_pool(name="sqt", bufs=1) as sqtp,
        tc.tile_pool(name="rcp", bufs=2) as rcpp,
        tc.tile_pool(name="pp", bufs=int(os.environ.get("KP_PP", 6)),
                     space="PSUM") as pp,
        tc.tile_pool(name="sqp", bufs=int(os.environ.get("KP_SQP", 1)),
                     space="PSUM") as sqp,
        tc.tile_pool(name="bcp", bufs=int(os.environ.get("KP_BCP", 1)),
                     space="PSUM") as bcp,
    ):
            xt_sb = xtp.tile([128, ND, TSH], F32R)

            # ---- projections: kT and qT (with RMS-norm), v (plain) ----
            def drain_ps(dst, ps, m, c0, cw, ssq):
                # raw copy (rounded to f32r) + square + ssq accumulation;
                # alternate engines to balance ACT vs DVE load
                if m % 2 == 0:
                    nc.scalar.copy(dst[:, m, c0:c0 + cw], ps[:, :cw])
                else:
                    nc.vector.tensor_copy(dst[:, m, c0:c0 + cw], ps[:, :cw])
                sq = scrp.tile([128, 512], F32R, tag="sq")
                if m % 2 == 0:
                    # DVE square must read the SBUF copy (one-PSUM-input rule)
                    nc.vector.tensor_mul(sq[:, :cw], dst[:, m, c0:c0 + cw],
                                         dst[:, m, c0:c0 + cw])
                else:
                    nc.scalar.square(sq[:, :cw], ps[:, :cw])
                nc.tensor.matmul(
                    ssq[:, :cw], onesc_sb[:], sq[:, :cw],
                    start=(m == 0), stop=(m == ND - 1),
                )

            def proj_normed(w_dram, dst, chunks, tok0, fold_qkw, first=False,
                            pool=None):
                pool = pool or pp
                """dst[:, m, c] = rstd * (x @ w)^T, rstd from raw sum-of-squares."""
                w_slices = []
                for kk in range(ND):
                    w_sl = wp.tile([128, DIM], F32R, tag="wslice")
                    nc.sync.dma_start(w_sl[:], w_dram[kk * 128:(kk + 1) * 128, :])
                    if first:
                        # interleave xt loads so the kk-outer first chunk can
                        # start as soon as the first (w, xt) slice pair lands
                        nc.sync.dma_start(xt_sb[:, kk, :], xt[:, kk, :])
                    w_slices.append(w_sl)
                for ci, (c0, cw) in enumerate(chunks):
                    ssq = sqp.tile([1, 512], F32)
                    if first and ci == 0:
                        # kk-outer in m-blocks of 4: PE consumes DMA'd slices
                        # incrementally instead of waiting for all 16
                        for mb in range(0, ND, 4):
                            blk = []
                            for m in range(mb, mb + 4):
                                ps = pool.tile([128, 512], F32, tag="ps")
                                blk.append(ps)
                            for kk in range(ND):
                                for mi, m in enumerate(range(mb, mb + 4)):
                                    nc.tensor.matmul(
                                        blk[mi][:, :cw],
                                        w_slices[kk][:, m * 128:(m + 1) * 128],
                                        xt_sb[:, kk, tok0 + c0: tok0 + c0 + cw],
                                        start=(kk == 0), stop=(kk == ND - 1),
                                    )
                            for mi, m in enumerate(range(mb, mb + 4)):
                                drain_ps(dst, blk[mi], m, c0, cw, ssq)
                    else:
                        for m in range(ND):
                            ps = pool.tile([128, 512], F32, tag="ps")
                            for kk in range(ND):
                                nc.tensor.matmul(
                                    ps[:, :cw],
                                    w_slices[kk][:, m * 128:(m + 1) * 128],
                                    xt_sb[:, kk, tok0 + c0: tok0 + c0 + cw],
                                    start=(kk == 0), stop=(kk == ND - 1),
                                )
                            drain_ps(dst, ps, m, c0, cw, ssq)
                    sqt = sqtp.tile([1, 512], F32, tag="sqt")
                    nc.scalar.activation(sqt[:, :cw], ssq[:, :cw], AF.Sqrt,
                                         bias=eps_sb[:], scale=1.0 / DIM)
                    rstd = rcpp.tile([1, 512], F32R, tag="rstd")
                    with nc.allow_low_precision(reason="f32r rstd for matmul"):
                        nc.vector.reciprocal(rstd[:, :cw], sqt[:, :cw])
                    if fold_qkw:
                        for m in range(ND):
                            rsb = bcp.tile([128, 512], F32)
                            nc.tensor.matmul(rsb[:, :cw], qkw_sb[:, m, :],
                                             rstd[:, :cw], start=True, stop=True)
                            nc.vector.tensor_mul(dst[:, m, c0:c0 + cw],
                                                 dst[:, m, c0:c0 + cw], rsb[:, :cw])
                    else:
                        rsb = bcp.tile([128, 512], F32)
                        nc.tensor.matmul(rsb[:, :cw], onesr_sb[:],
                                         rstd[:, :cw], start=True, stop=True)
                        for m in range(ND):
                            nc.vector.tensor_mul(dst[:, m, c0:c0 + cw],
                                                 dst[:, m, c0:c0 + cw], rsb[:, :cw])

            proj_normed(wk, kt_sb, K_CHUNKS, 0, fold_qkw=True, first=True)
            proj_normed(wq, qt_sb, Q_CHUNKS, HALO, fold_qkw=False)

            # v: natural layout [tok, dim]
            wv_slices = []
            for kk in range(ND):
                w_sl = wp.tile([128, DIM], F32R, tag="wslice")
                nc.sync.dma_start(w_sl[:], wv[kk * 128:(kk + 1) * 128, :])
                wv_slices.append(w_sl)
            for tt in range(NT):
                for nn in range(2):
                    ps = pp.tile([128, 512], F32)
                    for kk in range(ND):
                        nc.tensor.matmul(
                            ps[:],
                            xt_sb[:, kk, tt * 128:(tt + 1) * 128],
                            wv_slices[kk][:, nn * 512:(nn + 1) * 512],
                            start=(kk == 0), stop=(kk == ND - 1),
                        )
                    if tt % 2 == 0:
                        nc.scalar.copy(v_sb[:, tt, nn * 512:(nn + 1) * 512], ps[:])
                    else:
                        nc.vector.tensor_copy(v_sb[:, tt, nn * 512:(nn + 1) * 512],
                                              ps[:])


def self_phase2(tc, nc, kt_sb, qt_sb, v_sb, onesr_sb, onesc_sb,
                rel4_sb, wo, out):
        # xt freed; load wo and run attention + output projection
        with (
            tc.tile_pool(name="wo", bufs=1) as wop,
            tc.tile_pool(name="exp", bufs=int(os.environ.get("KP_EXP", 3))) as expp,
            tc.tile_pool(name="atc", bufs=int(os.environ.get("KP_ATC", 2))) as atcp,
            tc.tile_pool(name="outp", bufs=3) as outp,
            tc.tile_pool(name="rcp2", bufs=2) as rcp2p,
            tc.tile_pool(name="sps", bufs=int(os.environ.get("KP_SPS", 2)),
                         space="PSUM") as sps,
            tc.tile_pool(name="ytp", bufs=int(os.environ.get("KP_YTP", 1)),
                         space="PSUM") as ytp,
            tc.tile_pool(name="rsp", bufs=int(os.environ.get("KP_RSP", 1)),
                         space="PSUM") as rsp,
            tc.tile_pool(name="bc2", bufs=int(os.environ.get("KP_BC2", 1)),
                         space="PSUM") as bc2p,
            tc.tile_pool(name="pso", bufs=int(os.environ.get("KP_PSO", 1)),
                         space="PSUM") as psop,
        ):
            wo_sb = wop.tile([128, ND, DIM], F32R)
            for hd in range(ND):
                nc.sync.dma_start(wo_sb[:, hd, :], wo[hd * 128:(hd + 1) * 128, :])

            for qc in range(NQC):
                aT = atcp.tile([128, ND, QC], F32R)
                for h in range(H):
                    yT_t = ytp.tile([128, QC], F32, tag="yT")
                    rs_t = rsp.tile([1, QC], F32, tag="rs")
                    yT = yT_t[:, :]
                    rs = rs_t[:, :]
                    # joint [128, 3, QC] score tile: three QK matmuls, then ONE
                    # fused bias-add and ONE exp over all 768 columns.
                    # rel4 slots: [0]=j0-first-tile variant (per-core: all-NEG
                    # on first-half cores), [1]=j1, [2]=j2, [3]=j0-regular.
                    # qc=0 uses rel4[0:3] with slots (j0,j1,j2); qc>0 uses
                    # rel4[1:4] with slots (j1,j2,j0).
                    jmap = (0, 1, 2) if qc == 0 else (1, 2, 0)
                    rel_w = rel4_sb[:, 0:3, :] if qc == 0 else rel4_sb[:, 1:4, :]
                    stj = sps.tile([128, 3, QC], F32)
                    st = stj[:, 0:3, :]
                    for s, j in enumerate(jmap):
                        nc.tensor.matmul(
                            stj[:, s, :],
                            kt_sb[:, h, qc * QC + j * 128: qc * QC + (j + 1) * 128],
                            qt_sb[:, h, qc * QC: (qc + 1) * QC],
                            start=True, stop=True,
                        )
                    # scores += slope_h * rel (rel = -1e9 at masked positions)
                    nc.vector.scalar_tensor_tensor(
                        out=st[:], in0=rel_w, scalar=_SLOPES[h],
                        in1=st[:], op0=ALU.mult, op1=ALU.add)
                    ex = expp.tile([128, 3, QC], F32R, tag="exp")
                    nc.scalar.activation(ex[:], st[:], AF.Exp)
                    for s, j in enumerate(jmap):
                        nc.tensor.matmul(
                            yT,
                            v_sb[:, 2 * qc + j, h * 128:(h + 1) * 128],
                            ex[:, s, :], start=(s == 0), stop=(s == 2),
                        )
                        nc.tensor.matmul(
                            rs, onesc_sb[:], ex[:, s, :],
                            start=(s == 0), stop=(s == 2),
                        )
                    rcp = rcp2p.tile([1, QC], F32R, tag="rcp")
                    with nc.allow_low_precision(reason="f32r prob scale"):
                        nc.vector.reciprocal(rcp[:], rs)
                    rsb2_t = bc2p.tile([128, QC], F32, tag="rsb2")
                    rsb2 = rsb2_t[:, :]
                    nc.tensor.matmul(rsb2, onesr_sb[:], rcp[:],
                                     start=True, stop=True)
                    nc.scalar.copy(aT[:, h, :], yT)
                    nc.vector.tensor_mul(aT[:, h, :], aT[:, h, :], rsb2)

                # output projection for this q-chunk
                for t2 in range(QC // 128):
                    for nn in range(2):
                        ps_o = psop.tile([128, 512], F32)
                        for hd in range(ND):
                            nc.tensor.matmul(
                                ps_o[:],
                                aT[:, hd, t2 * 128:(t2 + 1) * 128],
                                wo_sb[:, hd, nn * 512:(nn + 1) * 512],
                                start=(hd == 0), stop=(hd == ND - 1),
                            )
                        o_sb = outp.tile([128, 512], F32, tag="osb")
                        nc.vector.tensor_copy(o_sb[:], ps_o[:])
                        nc.sync.dma_start(
                            out[qc * QC + t2 * 128: qc * QC + (t2 + 1) * 128,
                                nn * 512:(nn + 1) * 512],
                            o_sb[:],
                        )


def _host_constants():
    # relpat(j)[kj, qi] = 128*(j-1) + kj - qi if in window else NEG
    kj = np.arange(128)[:, None, None]
    jj = np.arange(3)[None, :, None]
    qi = np.arange(QC)[None, None, :]
    rel = 128 * (jj - 1) + kj - qi
    valid = (rel <= 0) & (rel >= -WINDOW)
    relpat = np.where(valid, rel, NEG).astype(np.float32)  # [128, 3, QC]
    ones_row = np.ones((1, 128), dtype=np.float32)
    ones_col = np.ones((128, 1), dtype=np.float32)
    return relpat, ones_row, ones_col


def _make_in_maps(x, wq, wk, wv, wo, q_norm_w, k_norm_w):
    x = np.ascontiguousarray(np.asarray(x, dtype=np.float32))
    wq = np.ascontiguousarray(np.asarray(wq, dtype=np.float32))
    wk = np.ascontiguousarray(np.asarray(wk, dtype=np.float32))
    wv = np.ascontiguousarray(np.asarray(wv, dtype=np.float32))
    wo = np.ascontiguousarray(np.asarray(wo, dtype=np.float32))
    q_norm_w = np.asarray(q_norm_w, dtype=np.float32)
    k_norm_w = np.asarray(k_norm_w, dtype=np.float32)

    relpat, ones_row, ones_col = _host_constants()
    qkw = (q_norm_w * k_norm_w / math.sqrt(HD)).astype(np.float32)
    qkw_row = qkw.reshape(1, ND, 128)

    in_maps = []
    for c in range(8):
        b, hf = c // 2, c % 2
        base = hf * (T // 2)
        xsh = np.zeros((TSH, DIM), dtype=np.float32)
        lo = base - HALO
        if lo < 0:
            xsh[HALO:] = x[b, base: base + QTOK]
        else:
            xsh[:] = x[b, lo: base + QTOK]
        xt_c = np.ascontiguousarray(
            xsh.T.reshape(ND, 128, TSH).transpose(1, 0, 2))
        rel4 = np.empty((128, 4, QC), dtype=np.float32)
        rel4[:, 1:3, :] = relpat[:, 1:3, :]          # j1, j2
        rel4[:, 3, :] = relpat[:, 0, :]              # j0 regular
        rel4[:, 0, :] = NEG if hf == 0 else relpat[:, 0, :]  # j0 first tile
        in_maps.append({
            "xt": xt_c, "wq": wq, "wk": wk, "wv": wv, "wo": wo,
            "qkw_row": qkw_row, "ones_row": ones_row, "ones_col": ones_col,
            "rel4": rel4,
        })

    return in_maps


def kernel(x, wq, wk, wv, wo, q_norm_w, k_norm_w):
    if "nc" not in _CACHE:
        _CACHE["nc"] = _build_program()
    nc = _CACHE["nc"]
    in_maps = _make_in_maps(x, wq, wk, wv, wo, q_norm_w, k_norm_w)
    _CACHE["in_maps"] = in_maps
    import time as _time
    last_err = None
    for attempt in range(3):
        try:
            res = run_bass_kernel_spmd(nc, in_maps, core_ids=list(range(8)))
            break
        except Exception as e:  # transient NRT/device wedges recover on retry
            last_err = e
            _time.sleep(10 * (attempt + 1))
    else:
        raise last_err

    out = np.empty((B, T, DIM), dtype=np.float32)
    for c in range(8):
        b, hf = c // 2, c % 2
        out[b, hf * QTOK:(hf + 1) * QTOK, :] = res.results[c]["out"]
    return out


# revision 53
# speedup vs baseline: 1.0317x; 1.0082x over previous
"""Trainium2 Bass kernel for nn_CodecAttention (sliding-window ALiBi attention).

Reference computation (B=4, T=2048, DIM=1024, H=8, HD=128, WINDOW=16):
    xq = rms_norm(x @ wq) ; xk = rms_norm(x @ wk) ; xv = x @ wv
    scores = q k^T / sqrt(HD) + alibi_bias  (causal + 16-token sliding window)
    out = softmax(scores) @ v  -> reshape -> @ wo

Sharding: 8 cores = (batch b, sequence half). Each core processes 1024 query
tokens plus a 128-token key/value halo (zeros for the first half), fully
locally -- the attention window (16) never crosses the halo, so no
collectives are needed.

Layout strategy (per core): everything transposed. Host passes xT [DIM, 1152].
Projections produce qT/kT in [dim, tok] layout and v in natural [tok, dim]
layout. Scores are computed transposed (sT[k, q] = kT.T @ qT per head), the
softmax denominator comes from a ones-column matmul (reduction over the
partition axis), and PV produces attn_outT[d, q] = v.T-free matmul with
exp(sT) as the moving operand. attn_outT is exactly the stationary operand the
wo matmul wants, so the final output lands in natural [tok, dim] layout with
zero transposes anywhere.

All matmuls run in float32r (full PE rate at N>=256, ~1.6e-4 rel err/K=128).
RMS norm: sum-of-squares via ones-matmul, rsqrt via ACT Sqrt + DVE reciprocal,
applied through a K=1 broadcast matmul (rstd per token broadcast across
partitions; the k-side broadcast uses q_norm_w*k_norm_w/sqrt(HD) as the
stationary operand, folding the norm weights and score scale in for free).

ALiBi + causal + window mask: tiny per-(j) rel tiles with -1e9 at invalid
positions; scores += slope_h * rel via one fused scalar_tensor_tensor. The
first key tile of the first q-chunk additionally subtracts a per-core
"negcol" column that kills out-of-range (global position < 0) halo keys.
"""

import math
import os

import numpy as np

os.environ.setdefault("MYCRO_LOCAL_CACHE", "1")

import concourse.mybir as mybir
import concourse.tile as tile
from concourse import bacc
from concourse.bass_utils import run_bass_kernel_spmd

F32 = mybir.dt.float32
F32R = mybir.dt.float32r
AF = mybir.ActivationFunctionType
ALU = mybir.AluOpType

B, T, DIM = 4, 2048, 1024
H, HD = 8, 128
WINDOW = 16
EPS = 1e-6
NEG = -1.0e9
BIGMASK = 30000.0

HALO = 128                 # key/value halo tokens per shard
TSH = HALO + T // 2        # 1152 tokens per shard
QTOK = T // 2              # 1024 query tokens per shard
ND = DIM // 128            # 8 dim tiles
NT = TSH // 128            # 9 token tiles
QC = 256                   # attention query-chunk width
NQC = QTOK // QC           # 4 query chunks
K_CHUNKS = [(0, 384), (384, 384), (768, 384)]        # kT projection chunks
Q_CHUNKS = [(0, 512), (512, 512)]                    # qT projection chunks

_SLOPES = [2.0 ** (-i) for i in range(H)]

_CACHE = {}


def _build_program():
    nc = bacc.Bacc("TRN2", debug=False, target_bir_lowering=False, num_devices=8)

    xt = nc.declare_dram_parameter("xt", [128, ND, TSH], F32R, isOutput=False)
    wq = nc.declare_dram_parameter("wq", [DIM, DIM], F32R, isOutput=False)
    wk = nc.declare_dram_parameter("wk", [DIM, DIM], F32R, isOutput=False)
    wv = nc.declare_dram_parameter("wv", [DIM, DIM], F32R, isOutput=False)
    wo = nc.declare_dram_parameter("wo", [DIM, DIM], F32R, isOutput=False)
    qkw_row = nc.declare_dram_parameter("qkw_row", [1, ND, 128], F32R, isOutput=False)
    ones_row = nc.declare_dram_parameter("ones_row", [1, 128], F32R, isOutput=False)
    ones_col = nc.declare_dram_parameter("ones_col", [128, 1], F32R, isOutput=False)
    rel4 = nc.declare_dram_parameter("rel4", [128, 4, QC], F32, isOutput=False)
    out = nc.declare_dram_parameter("out", [QTOK, DIM], F32, isOutput=True)

    with tile.TileContext(nc) as tc:
        with tc.tile_pool(name="big", bufs=1) as big:
            # ---- constants + persistent tensors (live for the whole kernel) ----
            kt_sb = big.tile([128, ND, TSH], F32R)
            qt_sb = big.tile([128, ND, QTOK], F32R)
            v_sb = big.tile([128, NT, DIM], F32R)
            qkw_sb = big.tile([1, ND, 128], F32R)
            onesr_sb = big.tile([1, 128], F32R)
            onesc_sb = big.tile([128, 1], F32R)
            rel4_sb = big.tile([128, 4, QC], F32)
            eps_sb = big.tile([1, 1], F32)
            nc.vector.memset(eps_sb[:], EPS)
            nc.sync.dma_start(qkw_sb[:], qkw_row[:])
            nc.sync.dma_start(onesr_sb[:], ones_row[:])
            nc.sync.dma_start(onesc_sb[:], ones_col[:])
            nc.sync.dma_start(rel4_sb[:], rel4[:])

            self_phase1(tc, nc, kt_sb, qt_sb, v_sb, qkw_sb, onesr_sb, onesc_sb,
                        eps_sb, xt, wq, wk, wv)
            self_phase2(tc, nc, kt_sb, qt_sb, v_sb, onesr_sb, onesc_sb,
                        rel4_sb, wo, out)
    nc.compile()
    return nc


def self_phase1(tc, nc, kt_sb, qt_sb, v_sb, qkw_sb, onesr_sb, onesc_sb,
                eps_sb, xt, wq, wk, wv):
    with (
        tc.tile_pool(name="xtp", bufs=1) as xtp,
        tc.tile_pool(name="wp", bufs=int(os.environ.get("KP_WP", 10))) as wp,
        tc.tile_pool(name="scr", bufs=2) as scrp,
        tc.tile_pool(name="sqt", bufs=1) as sqtp,
        tc.tile_pool(name="rcp", bufs=2) as rcpp,
        tc.tile_pool(name="pp", bufs=int(os.environ.get("KP_PP", 6)),
                     space="PSUM") as pp,
        tc.tile_pool(name="sqp", bufs=int(os.environ.get("KP_SQP", 1)),
                     space="PSUM") as sqp,
        tc.tile_pool(name="bcp", bufs=int(os.environ.get("KP_BCP", 1)),
                     space="PSUM") as bcp,
    ):
            xt_sb = xtp.tile([128, ND, TSH], F32R)

            # ---- projections: kT and qT (with RMS-norm), v (plain) ----
            def drain_ps(dst, ps, m, c0, cw, ssq):
                # raw copy (rounded to f32r) + square + ssq accumulation;
                # alternate engines to balance ACT vs DVE load
                if m % 2 == 0:
                    nc.scalar.copy(dst[:, m, c0:c0 + cw], ps[:, :cw])
                else:
                    nc.vector.tensor_copy(dst[:, m, c0:c0 + cw], ps[:, :cw])
                sq = scrp.tile([128, 512], F32R, tag="sq")
                if m % 2 == 0:
                    # DVE square must read the SBUF copy (one-PSUM-input rule)
                    nc.vector.tensor_mul(sq[:, :cw], dst[:, m, c0:c0 + cw],
                                         dst[:, m, c0:c0 + cw])
                else:
                    nc.scalar.square(sq[:, :cw], ps[:, :cw])
                nc.tensor.matmul(
                    ssq[:, :cw], onesc_sb[:], sq[:, :cw],
                    start=(m == 0), stop=(m == ND - 1),
                )

            def proj_normed(w_dram, dst, chunks, tok0, fold_qkw, first=False,
                            pool=None):
                pool = pool or pp
                """dst[:, m, c] = rstd * (x @ w)^T, rstd from raw sum-of-squares."""
                w_slices = []
                for kk in range(ND):
                    w_sl = wp.tile([128, DIM], F32R, tag="wslice")
                    nc.sync.dma_start(w_sl[:], w_dram[kk * 128:(kk + 1) * 128, :])
                    if first:
                        # interleave xt loads so the kk-outer first chunk can
                        # start as soon as the first (w, xt) slice pair lands
                        nc.sync.dma_start(xt_sb[:, kk, :], xt[:, kk, :])
                    w_slices.append(w_sl)
                for ci, (c0, cw) in enumerate(chunks):
                    ssq = sqp.tile([1, 512], F32)
                    if first and ci == 0:
                        # kk-outer in m-blocks of 4: PE consumes DMA'd slices
                        # incrementally instead of waiting for all 16
                        for mb in range(0, ND, 4):
                            blk = []
                            for m in range(mb, mb + 4):
                                ps = pool.tile([128, 512], F32, tag="ps")
                                blk.append(ps)
                            for kk in range(ND):
                                for mi, m in enumerate(range(mb, mb + 4)):
                                    nc.tensor.matmul(
                                        blk[mi][:, :cw],
                                        w_slices[kk][:, m * 128:(m + 1) * 128],
                                        xt_sb[:, kk, tok0 + c0: tok0 + c0 + cw],
                                        start=(kk == 0), stop=(kk == ND - 1),
                                    )
                            for mi, m in enumerate(range(mb, mb + 4)):
                                drain_ps(dst, blk[mi], m, c0, cw, ssq)
                    else:
                        for m in range(ND):
                            ps = pool.tile([128, 512], F32, tag="ps")
                            for kk in range(ND):
                                nc.tensor.matmul(
                                    ps[:, :cw],
                                    w_slices[kk][:, m * 128:(m + 1) * 128],
                                    xt_sb[:, kk, tok0 + c0: tok0 + c0 + cw],
                                    start=(kk == 0), stop=(kk == ND - 1),
                                )
                            drain_ps(dst, ps, m, c0, cw, ssq)
                    sqt = sqtp.tile([1, 512], F32, tag="sqt")
                    nc.scalar.activation(sqt[:, :cw], ssq[:, :cw], AF.Sqrt,
                                         bias=eps_sb[:], scale=1.0 / DIM)
                    rstd = rcpp.tile([1, 512], F32R, tag="rstd")
                    with nc.allow_low_precision(reason="f32r rstd for matmul"):
                        nc.vector.reciprocal(rstd[:, :cw], sqt[:, :cw])
                    if fold_qkw:
                        for m in range(ND):
                            rsb = bcp.tile([128, 512], F32)
                            nc.tensor.matmul(rsb[:, :cw], qkw_sb[:, m, :],
                                             rstd[:, :cw], start=True, stop=True)
                            nc.vector.tensor_mul(dst[:, m, c0:c0 + cw],
                                                 dst[:, m, c0:c0 + cw], rsb[:, :cw])
                    else:
                        rsb = bcp.tile([128, 512], F32)
                        nc.tensor.matmul(rsb[:, :cw], onesr_sb[:],
                                         rstd[:, :cw], start=True, stop=True)
                        # stage the broadcast in SBUF: frees the psum slot and
                        # keeps the 8 muls off the one-PSUM-operand path
                        rsb_sb = scrp.tile([128, 512], F32, tag="rsbsb")
                        nc.scalar.copy(rsb_sb[:, :cw], rsb[:, :cw])
                        for m in range(ND):
                            nc.vector.tensor_mul(dst[:, m, c0:c0 + cw],
                                                 dst[:, m, c0:c0 + cw],
                                                 rsb_sb[:, :cw])

            proj_normed(wk, kt_sb, K_CHUNKS, 0, fold_qkw=True, first=True)
            proj_normed(wq, qt_sb, Q_CHUNKS, HALO, fold_qkw=False)

            # v: natural layout [tok, dim]
            wv_slices = []
            for kk in range(ND):
                w_sl = wp.tile([128, DIM], F32R, tag="wslice")
                nc.sync.dma_start(w_sl[:], wv[kk * 128:(kk + 1) * 128, :])
                wv_slices.append(w_sl)
            for tt in range(NT):
                for nn in range(2):
                    ps = pp.tile([128, 512], F32)
                    for kk in range(ND):
                        nc.tensor.matmul(
                            ps[:],
                            xt_sb[:, kk, tt * 128:(tt + 1) * 128],
                            wv_slices[kk][:, nn * 512:(nn + 1) * 512],
                            start=(kk == 0), stop=(kk == ND - 1),
                        )
                    if tt % 2 == 0:
                        nc.scalar.copy(v_sb[:, tt, nn * 512:(nn + 1) * 512], ps[:])
                    else:
                        nc.vector.tensor_copy(v_sb[:, tt, nn * 512:(nn + 1) * 512],
                                              ps[:])


def self_phase2(tc, nc, kt_sb, qt_sb, v_sb, onesr_sb, onesc_sb,
                rel4_sb, wo, out):
        # xt freed; load wo and run attention + output projection
        with (
            tc.tile_pool(name="wo", bufs=1) as wop,
            tc.tile_pool(name="exp", bufs=int(os.environ.get("KP_EXP", 3))) as expp,
            tc.tile_pool(name="atc", bufs=int(os.environ.get("KP_ATC", 2))) as atcp,
            tc.tile_pool(name="outp", bufs=3) as outp,
            tc.tile_pool(name="rcp2", bufs=2) as rcp2p,
            tc.tile_pool(name="sps", bufs=int(os.environ.get("KP_SPS", 2)),
                         space="PSUM") as sps,
            tc.tile_pool(name="ytp", bufs=int(os.environ.get("KP_YTP", 1)),
                         space="PSUM") as ytp,
            tc.tile_pool(name="rsp", bufs=int(os.environ.get("KP_RSP", 1)),
                         space="PSUM") as rsp,
            tc.tile_pool(name="bc2", bufs=int(os.environ.get("KP_BC2", 1)),
                         space="PSUM") as bc2p,
            tc.tile_pool(name="pso", bufs=int(os.environ.get("KP_PSO", 1)),
                         space="PSUM") as psop,
        ):
            wo_sb = wop.tile([128, ND, DIM], F32R)
            for hd in range(ND):
                nc.sync.dma_start(wo_sb[:, hd, :], wo[hd * 128:(hd + 1) * 128, :])

            for qc in range(NQC):
                aT = atcp.tile([128, ND, QC], F32R)
                for h in range(H):
                    yT_t = ytp.tile([128, QC], F32, tag="yT")
                    rs_t = rsp.tile([1, QC], F32, tag="rs")
                    yT = yT_t[:, :]
                    rs = rs_t[:, :]
                    # joint [128, 3, QC] score tile: three QK matmuls, then ONE
                    # fused bias-add and ONE exp over all 768 columns.
                    # rel4 slots: [0]=j0-first-tile variant (per-core: all-NEG
                    # on first-half cores), [1]=j1, [2]=j2, [3]=j0-regular.
                    # qc=0 uses rel4[0:3] with slots (j0,j1,j2); qc>0 uses
                    # rel4[1:4] with slots (j1,j2,j0).
                    jmap = (0, 1, 2) if qc == 0 else (1, 2, 0)
                    rel_w = rel4_sb[:, 0:3, :] if qc == 0 else rel4_sb[:, 1:4, :]
                    stj = sps.tile([128, 3, QC], F32)
                    st = stj[:, 0:3, :]
                    for s, j in enumerate(jmap):
                        nc.tensor.matmul(
                            stj[:, s, :],
                            kt_sb[:, h, qc * QC + j * 128: qc * QC + (j + 1) * 128],
                            qt_sb[:, h, qc * QC: (qc + 1) * QC],
                            start=True, stop=True,
                        )
                    # scores += slope_h * rel (rel = -1e9 at masked positions)
                    nc.vector.scalar_tensor_tensor(
                        out=st[:], in0=rel_w, scalar=_SLOPES[h],
                        in1=st[:], op0=ALU.mult, op1=ALU.add)
                    ex = expp.tile([128, 3, QC], F32R, tag="exp")
                    nc.scalar.activation(ex[:], st[:], AF.Exp)
                    for s, j in enumerate(jmap):
                        nc.tensor.matmul(
                            yT,
                            v_sb[:, 2 * qc + j, h * 128:(h + 1) * 128],
                            ex[:, s, :], start=(s == 0), stop=(s == 2),
                        )
                        nc.tensor.matmul(
                            rs, onesc_sb[:], ex[:, s, :],
                            start=(s == 0), stop=(s == 2),
                        )
                    rcp = rcp2p.tile([1, QC], F32R, tag="rcp")
                    with nc.allow_low_precision(reason="f32r prob scale"):
                        nc.vector.reciprocal(rcp[:], rs)
                    rsb2_t = bc2p.tile([128, QC], F32, tag="rsb2")
                    rsb2 = rsb2_t[:, :]
                    nc.tensor.matmul(rsb2, onesr_sb[:], rcp[:],
                                     start=True, stop=True)
                    nc.scalar.copy(aT[:, h, :], yT)
                    nc.vector.tensor_mul(aT[:, h, :], aT[:, h, :], rsb2)

                # output projection for this q-chunk
                for t2 in range(QC // 128):
                    for nn in range(2):
                        ps_o = psop.tile([128, 512], F32)
                        for hd in range(ND):
                            nc.tensor.matmul(
                                ps_o[:],
                                aT[:, hd, t2 * 128:(t2 + 1) * 128],
                                wo_sb[:, hd, nn * 512:(nn + 1) * 512],
                                start=(hd == 0), stop=(hd == ND - 1),
                            )
                        o_sb = outp.tile([128, 512], F32, tag="osb")
                        nc.vector.tensor_copy(o_sb[:], ps_o[:])
                        nc.sync.dma_start(
                            out[qc * QC + t2 * 128: qc * QC + (t2 + 1) * 128,
                                nn * 512:(nn + 1) * 512],
                            o_sb[:],
                        )


def _host_constants():
    # relpat(j)[kj, qi] = 128*(j-1) + kj - qi if in window else NEG
    kj = np.arange(128)[:, None, None]
    jj = np.arange(3)[None, :, None]
    qi = np.arange(QC)[None, None, :]
    rel = 128 * (jj - 1) + kj - qi
    valid = (rel <= 0) & (rel >= -WINDOW)
    relpat = np.where(valid, rel, NEG).astype(np.float32)  # [128, 3, QC]
    ones_row = np.ones((1, 128), dtype=np.float32)
    ones_col = np.ones((128, 1), dtype=np.float32)
    return relpat, ones_row, ones_col


def _make_in_maps(x, wq, wk, wv, wo, q_norm_w, k_norm_w):
    x = np.ascontiguousarray(np.asarray(x, dtype=np.float32))
    wq = np.ascontiguousarray(np.asarray(wq, dtype=np.float32))
    wk = np.ascontiguousarray(np.asarray(wk, dtype=np.float32))
    wv = np.ascontiguousarray(np.asarray(wv, dtype=np.float32))
    wo = np.ascontiguousarray(np.asarray(wo, dtype=np.float32))
    q_norm_w = np.asarray(q_norm_w, dtype=np.float32)
    k_norm_w = np.asarray(k_norm_w, dtype=np.float32)

    relpat, ones_row, ones_col = _host_constants()
    qkw = (q_norm_w * k_norm_w / math.sqrt(HD)).astype(np.float32)
    qkw_row = qkw.reshape(1, ND, 128)

    in_maps = []
    for c in range(8):
        b, hf = c // 2, c % 2
        base = hf * (T // 2)
        xsh = np.zeros((TSH, DIM), dtype=np.float32)
        lo = base - HALO
        if lo < 0:
            xsh[HALO:] = x[b, base: base + QTOK]
        else:
            xsh[:] = x[b, lo: base + QTOK]
        xt_c = np.ascontiguousarray(
            xsh.T.reshape(ND, 128, TSH).transpose(1, 0, 2))
        rel4 = np.empty((128, 4, QC), dtype=np.float32)
        rel4[:, 1:3, :] = relpat[:, 1:3, :]          # j1, j2
        rel4[:, 3, :] = relpat[:, 0, :]              # j0 regular
        rel4[:, 0, :] = NEG if hf == 0 else relpat[:, 0, :]  # j0 first tile
        in_maps.append({
            "xt": xt_c, "wq": wq, "wk": wk, "wv": wv, "wo": wo,
            "qkw_row": qkw_row, "ones_row": ones_row, "ones_col": ones_col,
            "rel4": rel4,
        })

    return in_maps


def kernel(x, wq, wk, wv, wo, q_norm_w, k_norm_w):
    if "nc" not in _CACHE:
        _CACHE["nc"] = _build_program()
    nc = _CACHE["nc"]
    in_maps = _make_in_maps(x, wq, wk, wv, wo, q_norm_w, k_norm_w)
    _CACHE["in_maps"] = in_maps
    import time as _time
    last_err = None
    for attempt in range(3):
        try:
            res = run_bass_kernel_spmd(nc, in_maps, core_ids=list(range(8)))
            break
        except Exception as e:  # transient NRT/device wedges recover on retry
            last_err = e
            _time.sleep(10 * (attempt + 1))
    else:
        raise last_err

    out = np.empty((B, T, DIM), dtype=np.float32)
    for c in range(8):
        b, hf = c // 2, c % 2
        out[b, hf * QTOK:(hf + 1) * QTOK, :] = res.results[c]["out"]
    return out
